# revision 2
# baseline (speedup 1.0000x reference)
"""ColBERT MaxSim contrastive loss on 8 Trainium2 NeuronCores.

scores[b, c] = (1/q_len[b]) * sum_n max_s <q[b, n, :], d[c, s, :]>
loss = CE(scores / T, labels=arange(B)), mean reduction.

Sharding: data-parallel over the *doc* batch dim (columns of the score
matrix). Each core holds the full query set (1 MB) plus its 8-doc shard
(4 MB), computes its (B_global, B_local) = (64, 8) score block, and the
host performs the final gather + tiny 64x64 CE reduction.

v2 pipeline (trace-driven rewrite of the v1 kernel):
  The post-matmul max-reduction is the bottleneck: every PSUM element
  needs one first touch by ACT (copy->fp16, 1 elem/cyc @1.2 GHz) or DVE
  (reduce_max, 1 elem/cyc @0.96 GHz; fp16 tensor_tensor folds at 2x).
  v1 lost ~45 us at startup (group-major loop needed all 4 doc pairs
  before the first fold) and paid per-op overheads on 1024-wide tiles.

  v2: per group g, the pair r=g%4 is reduced *directly* off PSUM by DVE
  (reduce_max over a [128, 2, 1024] view, one op), and the other 3
  pairs are ACT-copied as whole [128, 2048] pair tiles (one ACTIVATE
  each, amortizing the ~322-cycle PSUM-read overhead) into a transient
  per-group staging tile, folded once by a 3-level fp16 tensor_max tree
  + one reduce_max. Rotating r over groups gives DVE work as soon as
  pair 0 lands, so compute ramps at ~6 us instead of ~50 us. PSUM pair
  tiles [128, 2048] (4 banks) x 2 bufs keep PE (55 us busy) pipelined
  against the ~2 us/unit drains.

Host: out blocks -> scores (64, 64) -> q_len scaling -> CE loss.
"""

import json

import numpy as np

import concourse.bass as bass
import concourse.mybir as mybir
import concourse.tile as tile
from concourse.bass_utils import run_bass_kernel_spmd

B = 64          # queries (= docs, contrastive batch)
NQ = 32         # tokens per query
ND = 1024       # tokens per doc
D = 128         # embedding dim
NCORES = 8
CL = B // NCORES  # docs per core
TEMPERATURE = 0.02
NORMALIZE_SCORES = True

F32 = mybir.dt.float32
F16 = mybir.dt.float16

NG = (B * NQ) // 128        # 16 query groups of 4 queries
NPAIR = CL // 2             # 4 doc pairs per core
NSETS = NG * CL             # 128 (query group, doc) sets


def _split_waits_json(bir_bytes: bytes) -> bytes:
    """Walrus in this toolchain rejects >1 sem-wait per instruction on the
    Tile end-of-kernel drain; split extra waits onto preceding Drains."""
    bir = json.loads(bir_bytes)
    for f in bir["functions"]:
        for blk in f["blocks"]:
            fixed = []
            for ins in blk["instructions"]:
                si = ins.get("sync_info") or {}
                waits = si.get("on_wait") or []
                if len(waits) > 1:
                    for i, w in enumerate(waits[:-1]):
                        fixed.append({
                            "debug": ins.get("debug", 0),
                            "engine": ins["engine"],
                            "ins": [],
                            "is_reset_sema": False,
                            "name": f'{ins["name"]}-wsplit{i}',
                            "opcode": "Drain",
                            "outs": [],
                            "sync_info": {"on_update": [], "on_wait": [w]},
                        })
                    si["on_wait"] = waits[-1:]
                    ins["sync_info"] = si
                fixed.append(ins)
            blk["instructions"] = fixed
    return json.dumps(bir).encode()


def _patch_nc(nc):
    orig = nc.to_json_bytes

    def patched(*a, **k):
        return _split_waits_json(orig(*a, **k))

    nc.to_json_bytes = patched
    return nc


def build_nc():
    """Build the per-core Bass program (SPMD: every core runs this; only
    the data in its "d" shard differs)."""
    nc = bass.Bass("TRN2", target_bir_lowering=False, debug=False,
                   num_devices=NCORES)
    q_dram = nc.dram_tensor("q", [B, NQ, D], F32, kind="ExternalInput").ap()
    d_dram = nc.dram_tensor("d", [CL, ND, D], F32, kind="ExternalInput").ap()
    sel_dram = nc.dram_tensor("sel", [128, 64], F16, kind="ExternalInput").ap()
    out_dram = nc.dram_tensor("out", [64, NSETS], F32, kind="ExternalOutput").ap()

    with tile.TileContext(nc) as tc:
        with (
            tc.tile_pool(name="prep", bufs=1) as prep,
            tc.tile_pool(name="qload", bufs=1) as qload_pool,
            tc.tile_pool(name="dnat", bufs=2) as dnat_pool,
            tc.tile_pool(name="d16", bufs=2) as d16_pool,
            tc.tile_pool(name="stg", bufs=2) as stg_pool,
            tc.tile_pool(name="fold", bufs=2) as fold_pool,
            tc.tile_pool(name="mm", bufs=2, space="PSUM") as psum_pool,
        ):
            # ---- q: one contiguous 1 MB load (8 KB descriptors).
            # Token tok = 16p + six lands on partition p of block six;
            # query b = p//2, so a 2-partition-group selector sums per
            # query and the host adds the 16 per-block partials. ----
            qT = prep.tile([128, NG * 128], F16)
            q_nat = qload_pool.tile([128, 2048], F32, tag="qn", name="qn")
            nc.scalar.dma_start(
                q_nat[:].rearrange("p (six d) -> p six d", six=16),
                q_dram.rearrange("bb n d -> (bb n) d").rearrange(
                    "(p six) d -> p six d", six=16))
            q16 = qload_pool.tile([128, 2048], F16, tag="q6", name="q6")
            nc.vector.tensor_copy(q16[:], q_nat[:])
            nc.sync.dma_start_transpose(
                qT[:].rearrange("p (six f) -> p six f", six=16), q16[:])

            # ---- d: per doc pair, 2 KB descriptors (8-token groups;
            # the in-block token permutation is fine for max) ----
            dT = []
            for p in range(NPAIR):
                d_nat = dnat_pool.tile([128, 2048], F32, tag="dnat",
                                       name="dnat")
                for c in range(2):
                    nc.scalar.dma_start(
                        d_nat[:, c * 1024:(c + 1) * 1024].rearrange(
                            "p (eight d) -> p eight d", eight=8),
                        d_dram[2 * p + c].rearrange(
                            "(p eight) d -> p eight d", eight=8),
                    )
                d16 = d16_pool.tile([128, 2048], F16, tag="d16", name="d16")
                nc.vector.tensor_copy(d16[:], d_nat[:])
                dTp = prep.tile([128, 2048], F16, tag=f"dT{p}", name=f"dT{p}")
                nc.sync.dma_start_transpose(
                    dTp[:].rearrange("p (t f) -> p t f", t=16), d16[:])
                dT.append(dTp)

            # selector: sel[p, mm] = 1 if p//2 == mm (2 tokens per query
            # land in each partition group per block)
            sel = prep.tile([128, 64], F16)
            nc.scalar.dma_start(sel[:], sel_dram)

            # fp16 so DVE ops on it keep their packed modes
            maxes = prep.tile([128, NSETS], F16)

            # ---- main loop: 16 query groups x 4 doc pairs ----
            # Per group: pair r=g%4 is DVE-direct off PSUM; the other 3
            # pairs are ACT-staged to fp16 and tree-folded by DVE.
            for g in range(NG):
                r = g % NPAIR
                lhs = qT[:, bass.ts(g, 128)]
                stg = stg_pool.tile([128, 3 * 2048], F16, tag="stg",
                                    name="stg")
                k = 0
                for p in range(NPAIR):
                    pt = psum_pool.tile([128, 2048], F32, tag="mm", name="mm")
                    for c in range(2):
                        rhs = dT[p][:, c * 1024:(c + 1) * 1024]
                        base = c * 1024
                        nc.tensor.matmul(pt[:, base:base + 512], lhs,
                                         rhs[:, 0:512], start=True, stop=True)
                        nc.tensor.matmul(pt[:, base + 512:base + 1024], lhs,
                                         rhs[:, 512:1024], start=True,
                                         stop=True)
                    if p == r:
                        # direct: per-doc max over the [128, 2, 1024] view
                        nc.vector.reduce_max(
                            maxes[:, g * CL + 6:g * CL + 8],
                            pt[:].rearrange("p (s f) -> p s f", s=2),
                            axis=mybir.AxisListType.X)
                    else:
                        nc.scalar.copy(stg[:, bass.ts(k, 2048)], pt[:])
                        k += 1
                # fold the 6 staged docs: fp16 TT(max) tree at 2x, then
                # one 1x reduce_max of the 128-wide remainders
                st1 = fold_pool.tile([128, 6 * 512], F16, tag="st1",
                                     name="st1")
                st2 = fold_pool.tile([128, 6 * 256], F16, tag="st2",
                                     name="st2")
                st3 = fold_pool.tile([128, 6 * 128], F16, tag="st3",
                                     name="st3")
                v0 = stg[:].rearrange("p (s f) -> p s f", s=6)
                v1 = st1[:].rearrange("p (s f) -> p s f", s=6)
                v2 = st2[:].rearrange("p (s f) -> p s f", s=6)
                v3 = st3[:].rearrange("p (s f) -> p s f", s=6)
                nc.vector.tensor_max(out=v1, in0=v0[:, :, 0:512],
                                     in1=v0[:, :, 512:1024])
                nc.vector.tensor_max(out=v2, in0=v1[:, :, 0:256],
                                     in1=v1[:, :, 256:512])
                nc.vector.tensor_max(out=v3, in0=v2[:, :, 0:128],
                                     in1=v2[:, :, 128:256])
                nc.vector.reduce_max(maxes[:, g * CL:g * CL + 6], v3,
                                     axis=mybir.AxisListType.X)

            # ---- reduce over the 32 tokens of each query ----
            sel_ps = psum_pool.tile([64, NSETS], F32, tag="mm", name="selps")
            nc.tensor.matmul(sel_ps[:], sel[:], maxes[:], start=True, stop=True)
            out_sb = prep.tile([64, NSETS], F32)
            nc.vector.tensor_copy(out_sb[:], sel_ps[:])
            nc.sync.dma_start(out_dram, out_sb[:])

    nc.finalize()
    return _patch_nc(nc)


_NC = None


def _get_nc():
    global _NC
    if _NC is None:
        _NC = build_nc()
    return _NC


def assemble_loss(outs, q):
    """Host tail: per-core [64, 128] blocks -> scores -> CE loss.

    blk[b, g*8 + j] is the partial score (2 query tokens of six-block g)
    of query b against local doc perm(g, j): slots 0..5 are the staged
    pairs (p != g%4, ascending), slots 6..7 the direct pair p = g%4."""
    perm = np.zeros((NG, CL), np.int64)
    for g in range(NG):
        r = g % NPAIR
        staged = [p for p in range(NPAIR) if p != r]
        order = [2 * p + c for p in staged for c in range(2)] + [2 * r, 2 * r + 1]
        for j, doc in enumerate(order):
            perm[g, j] = doc
    scores = np.zeros((B, B), np.float64)
    for k in range(NCORES):
        blk = np.asarray(outs[k], np.float64).reshape(B, NG, CL)
        acc = np.zeros((B, CL), np.float64)
        for g in range(NG):
            acc[:, perm[g]] += blk[:, g, :]
        scores[:, CL * k:CL * (k + 1)] = acc
    if NORMALIZE_SCORES:
        q_len = (np.asarray(q)[:, :, 0] != 0).sum(axis=1).astype(np.float64)
        scores = scores / q_len[:, None]
    logits = scores / TEMPERATURE
    m = logits.max(axis=1, keepdims=True)
    logz = m[:, 0] + np.log(np.exp(logits - m).sum(axis=1))
    loss = -(np.diag(logits) - logz).mean()
    return np.float32(loss)


def make_sel():
    sel = np.zeros((128, 64), np.float16)
    for m in range(64):
        sel[2 * m:2 * (m + 1), m] = 1.0
    return sel


def kernel(query_embeddings, doc_embeddings):
    q = np.ascontiguousarray(np.asarray(query_embeddings, dtype=np.float32))
    d = np.ascontiguousarray(np.asarray(doc_embeddings, dtype=np.float32))
    nc = _get_nc()
    sel = make_sel()
    in_maps = [
        {"q": q, "d": np.ascontiguousarray(d[CL * k:CL * (k + 1)]),
         "sel": sel}
        for k in range(NCORES)
    ]
    res = run_bass_kernel_spmd(nc, in_maps, core_ids=list(range(NCORES)))
    outs = [res.results[k]["out"] for k in range(NCORES)]
    return assemble_loss(outs, q)


# revision 3
# speedup vs baseline: 1.0971x; 1.0971x over previous
"""ColBERT MaxSim contrastive loss on 8 Trainium2 NeuronCores.

scores[b, c] = (1/q_len[b]) * sum_n max_s <q[b, n, :], d[c, s, :]>
loss = CE(scores / T, labels=arange(B)), mean reduction.

Sharding: data-parallel over the *doc* batch dim (columns of the score
matrix). Each core holds the full query set (1 MB) plus its 8-doc shard
(4 MB), computes its (B_global, B_local) = (64, 8) score block, and the
host performs the final gather + tiny 64x64 CE reduction.

v3 (trace-driven):
  The post-matmul max-reduction is the bottleneck: every PSUM element
  needs one first touch by ACT (copy->fp16, ~1 elem/cyc @1.2 GHz) or
  DVE (reduce_max, ~1 elem/cyc @0.96 GHz; fp16 tensor_max folds at 2x).
  v1's steady state already saturated ACT (107% busy) with 4 rotating
  [128, 1024] PSUM slots, but lost ~45 us at startup: the d pair-load
  chains shared 2 pool slots, serializing load->cast->transpose per
  pair (~9 us each), and every group's fold needs 3 pairs.

  v3 keeps the v1 steady-state structure (4 PSUM slots, group-major
  loop, per-group batched fold tree) and fixes the ramp + balance:
   - every d pair gets dedicated load/cast tiles -> all 9 input DMAs
     issue at ~7 us and pipeline through the DMA rings;
   - q is loaded with one contiguous 8 KB descriptor per partition
     (the token permutation this induces is absorbed by the same
     transpose + selector structure);
   - per group, one *pair* (rotating r = g % 4) is DVE-direct-reduced
     off PSUM and 3 pairs are ACT-staged (n_dir=2 uniform, the
     measured ACT/DVE balance point); the rotation gives DVE direct
     work as soon as pair 0 lands;
   - staging pool holds 3 groups so folds overlap the next groups.

Host: out blocks -> scores (64, 64) -> q_len scaling -> CE loss.
"""

import json

import numpy as np

import concourse.bass as bass
import concourse.mybir as mybir
import concourse.tile as tile
from concourse.bass_utils import run_bass_kernel_spmd

B = 64          # queries (= docs, contrastive batch)
NQ = 32         # tokens per query
ND = 1024       # tokens per doc
D = 128         # embedding dim
NCORES = 8
CL = B // NCORES  # docs per core
TEMPERATURE = 0.02
NORMALIZE_SCORES = True

F32 = mybir.dt.float32
F16 = mybir.dt.float16

NG = (B * NQ) // 128        # 16 query groups of 4 queries
NPAIR = CL // 2             # 4 doc pairs per core
NSETS = NG * CL             # 128 (query group, doc) sets


def _split_waits_json(bir_bytes: bytes) -> bytes:
    """Walrus in this toolchain rejects >1 sem-wait per instruction on the
    Tile end-of-kernel drain; split extra waits onto preceding Drains."""
    bir = json.loads(bir_bytes)
    for f in bir["functions"]:
        for blk in f["blocks"]:
            fixed = []
            for ins in blk["instructions"]:
                si = ins.get("sync_info") or {}
                waits = si.get("on_wait") or []
                if len(waits) > 1:
                    for i, w in enumerate(waits[:-1]):
                        fixed.append({
                            "debug": ins.get("debug", 0),
                            "engine": ins["engine"],
                            "ins": [],
                            "is_reset_sema": False,
                            "name": f'{ins["name"]}-wsplit{i}',
                            "opcode": "Drain",
                            "outs": [],
                            "sync_info": {"on_update": [], "on_wait": [w]},
                        })
                    si["on_wait"] = waits[-1:]
                    ins["sync_info"] = si
                fixed.append(ins)
            blk["instructions"] = fixed
    return json.dumps(bir).encode()


def _patch_nc(nc):
    orig = nc.to_json_bytes

    def patched(*a, **k):
        return _split_waits_json(orig(*a, **k))

    nc.to_json_bytes = patched
    return nc


def build_nc():
    """Build the per-core Bass program (SPMD: every core runs this; only
    the data in its "d" shard differs)."""
    nc = bass.Bass("TRN2", target_bir_lowering=False, debug=False,
                   num_devices=NCORES)
    q_dram = nc.dram_tensor("q", [B, NQ, D], F32, kind="ExternalInput").ap()
    d_dram = nc.dram_tensor("d", [CL, ND, D], F32, kind="ExternalInput").ap()
    sel_dram = nc.dram_tensor("sel", [128, 64], F16, kind="ExternalInput").ap()
    out_dram = nc.dram_tensor("out", [64, NSETS], F32, kind="ExternalOutput").ap()

    with tile.TileContext(nc) as tc:
        with (
            tc.tile_pool(name="prep", bufs=1) as prep,
            tc.tile_pool(name="stg", bufs=3) as stg_pool,
            tc.tile_pool(name="fold", bufs=2) as fold_pool,
            tc.tile_pool(name="mm", bufs=4, space="PSUM") as psum_pool,
        ):
            # ---- q: one contiguous load (8 KB per-partition descriptors;
            # partition p holds tokens 16p..16p+15). After the blockwise
            # transpose, block g of qT holds tokens {16j + g}, so query
            # b = j//2 lands on partition pair (2b, 2b+1) of every block
            # and the same 2-partition selector + host 16-block sum
            # applies. ----
            qT = prep.tile([128, NG * 128], F16)
            q_nat = prep.tile([128, 2048], F32, tag="qn", name="qn")
            nc.scalar.dma_start(
                q_nat[:].rearrange("p (t d) -> p t d", t=16),
                q_dram.rearrange("bb n d -> (bb n) d").rearrange(
                    "(p t) d -> p t d", t=16))
            q16 = prep.tile([128, 2048], F16, tag="q6", name="q6")
            nc.vector.tensor_copy(q16[:], q_nat[:])
            nc.sync.dma_start_transpose(
                qT[:].rearrange("p (t f) -> p t f", t=16), q16[:])

            # ---- d: per doc pair, 4 KB per-partition descriptors
            # (partition p holds tokens 8p..8p+7 of each doc; the
            # in-block token permutation is fine for max). Dedicated
            # tiles per pair so all loads issue immediately and the
            # load->cast->transpose chains pipeline. ----
            dT = []
            for p in range(NPAIR):
                d_nat = prep.tile([128, 2048], F32, tag=f"dn{p}",
                                  name=f"dn{p}")
                for c in range(2):
                    nc.scalar.dma_start(
                        d_nat[:, c * 1024:(c + 1) * 1024].rearrange(
                            "p (eight d) -> p eight d", eight=8),
                        d_dram[2 * p + c].rearrange(
                            "(p eight) d -> p eight d", eight=8),
                    )
                d16 = prep.tile([128, 2048], F16, tag=f"d6{p}",
                                name=f"d6{p}")
                nc.vector.tensor_copy(d16[:], d_nat[:])
                dTp = prep.tile([128, 2048], F16, tag=f"dT{p}", name=f"dT{p}")
                nc.sync.dma_start_transpose(
                    dTp[:].rearrange("p (t f) -> p t f", t=16), d16[:])
                dT.append(dTp)

            # selector: sel[p, mm] = 1 if p//2 == mm (2 tokens per query
            # land in each partition group per block)
            sel = prep.tile([128, 64], F16)
            nc.scalar.dma_start(sel[:], sel_dram)

            # fp16 so DVE ops on it keep their packed modes
            maxes = prep.tile([128, NSETS], F16)

            # ---- main loop: 16 query groups; per group the pair g%4 is
            # DVE-direct, the other 3 pairs are ACT-staged + tree-folded.
            for g in range(NG):
                r = g % NPAIR
                lhs = qT[:, bass.ts(g, 128)]
                stg = stg_pool.tile([128, 6 * 1024], F16, tag="stg",
                                    name="stg")
                k = 0
                for p in [r] + [x for x in range(NPAIR) if x != r]:
                    for c in range(2):
                        pa = psum_pool.tile([128, 1024], F32, tag="pa",
                                            name="pa")
                        rhs = dT[p][:, c * 1024:(c + 1) * 1024]
                        nc.tensor.matmul(pa[:, 0:512], lhs, rhs[:, 0:512],
                                         start=True, stop=True)
                        nc.tensor.matmul(pa[:, 512:1024], lhs,
                                         rhs[:, 512:1024], start=True,
                                         stop=True)
                        if p == r:
                            nc.vector.reduce_max(
                                maxes[:, g * CL + 6 + c:g * CL + 7 + c],
                                pa[:], axis=mybir.AxisListType.X)
                        else:
                            nc.scalar.copy(stg[:, bass.ts(k, 1024)], pa[:])
                            k += 1
                # fold the 6 staged docs: fp16 TT(max) tree at 2x, then
                # one 1x reduce_max of the 128-wide remainders
                st1 = fold_pool.tile([128, 6 * 512], F16, tag="st1",
                                     name="st1")
                st2 = fold_pool.tile([128, 6 * 256], F16, tag="st2",
                                     name="st2")
                st3 = fold_pool.tile([128, 6 * 128], F16, tag="st3",
                                     name="st3")
                v0 = stg[:].rearrange("p (s f) -> p s f", s=6)
                v1 = st1[:].rearrange("p (s f) -> p s f", s=6)
                v2 = st2[:].rearrange("p (s f) -> p s f", s=6)
                v3 = st3[:].rearrange("p (s f) -> p s f", s=6)
                nc.vector.tensor_max(out=v1, in0=v0[:, :, 0:512],
                                     in1=v0[:, :, 512:1024])
                nc.vector.tensor_max(out=v2, in0=v1[:, :, 0:256],
                                     in1=v1[:, :, 256:512])
                nc.vector.tensor_max(out=v3, in0=v2[:, :, 0:128],
                                     in1=v2[:, :, 128:256])
                nc.vector.reduce_max(maxes[:, g * CL:g * CL + 6], v3,
                                     axis=mybir.AxisListType.X)

            # ---- reduce over the 32 tokens of each query ----
            sel_ps = psum_pool.tile([64, NSETS], F32, tag="pa", name="selps")
            nc.tensor.matmul(sel_ps[:], sel[:], maxes[:], start=True, stop=True)
            out_sb = prep.tile([64, NSETS], F32)
            nc.vector.tensor_copy(out_sb[:], sel_ps[:])
            nc.sync.dma_start(out_dram, out_sb[:])

    nc.finalize()
    return _patch_nc(nc)


_NC = None


def _get_nc():
    global _NC
    if _NC is None:
        _NC = build_nc()
    return _NC


def assemble_loss(outs, q):
    """Host tail: per-core [64, 128] blocks -> scores -> CE loss.

    blk[b, g*8 + j] is the partial score (2 query tokens of block g)
    of query b against local doc perm(g, j): slots 0..5 are the staged
    pairs (p != g%4, ascending), slots 6..7 the direct pair p = g%4."""
    perm = np.zeros((NG, CL), np.int64)
    for g in range(NG):
        r = g % NPAIR
        staged = [p for p in range(NPAIR) if p != r]
        order = [2 * p + c for p in staged for c in range(2)] + [2 * r, 2 * r + 1]
        for j, doc in enumerate(order):
            perm[g, j] = doc
    scores = np.zeros((B, B), np.float64)
    for k in range(NCORES):
        blk = np.asarray(outs[k], np.float64).reshape(B, NG, CL)
        acc = np.zeros((B, CL), np.float64)
        for g in range(NG):
            acc[:, perm[g]] += blk[:, g, :]
        scores[:, CL * k:CL * (k + 1)] = acc
    if NORMALIZE_SCORES:
        q_len = (np.asarray(q)[:, :, 0] != 0).sum(axis=1).astype(np.float64)
        scores = scores / q_len[:, None]
    logits = scores / TEMPERATURE
    m = logits.max(axis=1, keepdims=True)
    logz = m[:, 0] + np.log(np.exp(logits - m).sum(axis=1))
    loss = -(np.diag(logits) - logz).mean()
    return np.float32(loss)


def make_sel():
    sel = np.zeros((128, 64), np.float16)
    for m in range(64):
        sel[2 * m:2 * (m + 1), m] = 1.0
    return sel


def kernel(query_embeddings, doc_embeddings):
    q = np.ascontiguousarray(np.asarray(query_embeddings, dtype=np.float32))
    d = np.ascontiguousarray(np.asarray(doc_embeddings, dtype=np.float32))
    nc = _get_nc()
    sel = make_sel()
    in_maps = [
        {"q": q, "d": np.ascontiguousarray(d[CL * k:CL * (k + 1)]),
         "sel": sel}
        for k in range(NCORES)
    ]
    res = run_bass_kernel_spmd(nc, in_maps, core_ids=list(range(NCORES)))
    outs = [res.results[k]["out"] for k in range(NCORES)]
    return assemble_loss(outs, q)
